# revision 17
# baseline (speedup 1.0000x reference)
"""Trainium2 Bass kernel: parameter-distribution KL (DPO-style) loss.

Computes, for P=4 parameter rows of N=16.7M fp32 elements each:
    z = (x - mean) / std(ddof=1)   per row, both tensors
    p = softmax(z)
    kl_r = sum(p_init * (log p_init - log(p_cur + eps)))
    out = -(sum_r kl_r) / P        (fp32 scalar)

Distribution: flat axis N sharded across 8 NeuronCores, ZERO collectives.
Each core normalizes with its own shard statistics (local mean/std agree
with global to ~5e-4); the host reassembles the global softmax/KL in
float64 with first-order corrections (method error ~1e-6, validated).
Each core reads its HBM shard exactly once and streams continuously.

Device per core, per row (all with LOCAL shard stats):
  cur phase:  bn stats -> a_c,b_c;  w = exp(a_c*xc + b_c) (accum -> Sc)
              wbias = eps*8*Sc_local;  w = ln(w + wbias)   (in-place)
  init phase: bn stats -> a_i,b_i;  u = exp(a_i*xi + b_i) (accum -> Si)
              Q  += diag Gram(u, bf16(xi));  R += diag Gram(u, w)
              (PE, PSUM accumulate; diagonals extracted via identity mask)
Host (float64): per-core sums -> alpha/beta corrections -> global Si, Sc, T;
  kl = T/Si + ln Sc - ln Si.
"""

import numpy as np

P = 4
N = 16777216
NCORES = 8
SHARD = N // NCORES          # 2097152 elements per row per core
F = SHARD // 128             # 16384 free elems per partition
UNITS = 8
EPS = 1e-8

_cache = {}


def _build(F=F, UNITS=UNITS, N=N):
    FU = F // UNITS
    BN_CH = FU // 512
    SH = 128 * F             # local shard size
    import concourse.bacc as bacc
    import concourse.bass_isa as bass_isa
    import concourse.tile as tile
    import concourse.mybir as mybir

    fp32 = mybir.dt.float32
    bf16 = mybir.dt.bfloat16
    AF = mybir.ActivationFunctionType
    OP = mybir.AluOpType
    AX = mybir.AxisListType

    nc = bacc.Bacc("TRN2", target_bir_lowering=False, debug=False,
                   num_devices=NCORES)

    xi_dram = nc.dram_tensor("xi", [P, 128, F], fp32, kind="ExternalInput").ap()
    xc_dram = nc.dram_tensor("xc", [P, 128, F], fp32, kind="ExternalInput").ap()
    id_dram = nc.dram_tensor("ident", [128, 128], bf16,
                             kind="ExternalInput").ap()
    # per row: [128, 8] = [sumi_p, ssqi_p, sumc_p, ssqc_p, q_p, r_p, si_p, sc_p]
    stats_dram = nc.dram_tensor("stats", [P, 128, 8], fp32,
                                kind="ExternalOutput").ap()

    with tile.TileContext(nc) as tc:
        with tc.tile_pool(name="xpool", bufs=8) as xpool, \
             tc.tile_pool(name="bfpool", bufs=3) as bfpool, \
             tc.tile_pool(name="bnpool", bufs=2) as bnpool, \
             tc.tile_pool(name="accpool", bufs=2) as accpool, \
             tc.tile_pool(name="small", bufs=2) as small, \
             tc.tile_pool(name="psum", bufs=2, space="PSUM") as psum:

            ident = small.tile([128, 128], bf16, tag="ident", bufs=1,
                               name="ident")
            nc.sync.dma_start(ident[:], id_dram[:])

            def local_ab(r, x_dram_t, side):
                """Load one tensor of row r, bn stats, local a/b from the
                shard's own statistics. Returns (x_tiles, partials, ab)."""
                x_ts = []
                bn_t = bnpool.tile([128, UNITS * BN_CH, 6], fp32,
                                   tag=f"bn{side}", name=f"bn{side}{r}")
                for k in range(UNITS):
                    x_t = xpool.tile([128, FU], fp32, tag=f"x{side}",
                                     name=f"x{side}{r}_{k}")
                    nc.sync.dma_start(x_t[:], x_dram_t[:, k * FU:(k + 1) * FU])
                    for j in range(BN_CH):
                        idx = k * BN_CH + j
                        nc.vector.bn_stats(bn_t[:, idx:idx + 1, :],
                                           x_t[:, j * 512:(j + 1) * 512])
                    x_ts.append(x_t)
                aggr = small.tile([128, 2], fp32, tag=f"aggr{side}",
                                  name=f"ag{side}{r}")
                nc.vector.bn_aggr(aggr[:], bn_t[:])

                partials = small.tile([128, 2], fp32, tag=f"part{side}",
                                      name=f"pt{side}{r}")
                msq = small.tile([128, 1], fp32, tag=f"msq{side}",
                                 name=f"msq{side}{r}")
                nc.vector.tensor_mul(msq[:], aggr[:, 0:1], aggr[:, 0:1])
                nc.vector.tensor_scalar_mul(partials[:, 0:1], aggr[:, 0:1],
                                            float(F))
                nc.vector.tensor_scalar(partials[:, 1:2], aggr[:, 1:2],
                                        msq[:], float(F),
                                        op0=OP.add, op1=OP.mult)
                par = small.tile([128, 2], fp32, tag=f"par{side}",
                                 name=f"par{side}{r}")
                nc.gpsimd.partition_all_reduce(par[:], partials[:],
                                               channels=128,
                                               reduce_op=bass_isa.ReduceOp.add)
                # a = var^-0.5 = exp(-0.5*ln var), b = -mean*a
                ab = small.tile([128, 2], fp32, tag=f"ab{side}",
                                name=f"ab{side}{r}")
                tmp = small.tile([128, 3], fp32, tag=f"tmp{side}",
                                 name=f"tm{side}{r}")
                mean, prod, lnv = tmp[:, 0:1], tmp[:, 1:2], tmp[:, 2:3]
                nc.vector.tensor_scalar_mul(mean, par[:, 0:1], 1.0 / SH)
                nc.vector.tensor_mul(prod, par[:, 0:1], mean)
                nc.vector.tensor_scalar(lnv, par[:, 1:2], prod, 1.0 / (SH - 1),
                                        op0=OP.subtract, op1=OP.mult)
                nc.scalar.activation(lnv, lnv, AF.Ln)
                nc.scalar.activation(ab[:, 0:1], lnv, AF.Exp, scale=-0.5)
                nc.vector.tensor_scalar(ab[:, 1:2], ab[:, 0:1], mean, -1.0,
                                        op0=OP.mult, op1=OP.mult)
                return x_ts, partials, ab

            for r in range(P):
                # ---------------- cur phase ----------------
                xc_ts, part_c, ab_c = local_ab(r, xc_dram[r], "c")
                a_c, b_c = ab_c[:, 0:1], ab_c[:, 1:2]
                scacc = accpool.tile([128, UNITS], fp32, tag="scacc",
                                     name=f"sc{r}")
                w_ts = []
                for k in range(UNITS):
                    w_t = bfpool.tile([128, FU], bf16, tag="w",
                                      name=f"w{r}_{k}", bufs=12)
                    nc.scalar.activation(w_t[:], xc_ts[k][:], AF.Exp,
                                         bias=b_c, scale=a_c,
                                         accum_out=scacc[:, k:k + 1])
                    w_ts.append(w_t)
                scrow = small.tile([128, 1], fp32, tag="scrow", name=f"scr{r}")
                nc.vector.tensor_reduce(scrow[:], scacc[:], axis=AX.X,
                                        op=OP.add)
                par2 = small.tile([128, 1], fp32, tag="par2", name=f"par2{r}")
                nc.gpsimd.partition_all_reduce(par2[:], scrow[:],
                                               channels=128,
                                               reduce_op=bass_isa.ReduceOp.add)
                wbias = small.tile([128, 1], fp32, tag="wbias", name=f"wb{r}")
                nc.vector.tensor_scalar_mul(wbias[:], par2[:],
                                            EPS * NCORES)
                for k in range(UNITS):
                    nc.scalar.activation(w_ts[k][:], w_ts[k][:], AF.Ln,
                                         bias=wbias[:], scale=1.0)

                # ---------------- init phase ----------------
                xi_ts, part_i, ab_i = local_ab(r, xi_dram[r], "i")
                a_i, b_i = ab_i[:, 0:1], ab_i[:, 1:2]
                siacc = accpool.tile([128, UNITS], fp32, tag="siacc",
                                     name=f"si{r}")
                gram_q = psum.tile([128, 128], fp32, tag="gq", name=f"gq{r}")
                gram_r = psum.tile([128, 128], fp32, tag="gr", name=f"gr{r}")
                nchunk = FU // 128
                for k in range(UNITS):
                    u_t = bfpool.tile([128, FU], bf16, tag="u",
                                      name=f"u{r}_{k}", bufs=3)
                    xb_t = bfpool.tile([128, FU], bf16, tag="xb",
                                       name=f"xb{r}_{k}", bufs=3)
                    nc.scalar.activation(u_t[:], xi_ts[k][:], AF.Exp,
                                         bias=b_i, scale=a_i,
                                         accum_out=siacc[:, k:k + 1])
                    nc.vector.tensor_copy(xb_t[:], xi_ts[k][:])
                    for c in range(nchunk):
                        sl = slice(c * 128, (c + 1) * 128)
                        first = (k == 0 and c == 0)
                        last = (k == UNITS - 1 and c == nchunk - 1)
                        nc.tensor.matmul(gram_q[:], u_t[:, sl],
                                         xb_t[:, sl],
                                         start=first, stop=last)
                        nc.tensor.matmul(gram_r[:], u_t[:, sl],
                                         w_ts[k][:, sl],
                                         start=first, stop=last)

                # ---------------- row outputs ----------------
                accrow = accpool.tile([128, 4], fp32, tag="accrow",
                                      name=f"ar{r}")
                dscr = small.tile([128, 128], bf16, tag="dscr", name=f"ds{r}")
                nc.vector.scalar_tensor_tensor(
                    dscr[:], gram_r[:], 1.0, ident[:], OP.mult, OP.mult,
                    accum_out=accrow[:, 1:2])
                dscr2 = small.tile([128, 128], bf16, tag="dscr2",
                                   name=f"ds2{r}")
                nc.vector.scalar_tensor_tensor(
                    dscr2[:], gram_q[:], 1.0, ident[:], OP.mult, OP.mult,
                    accum_out=accrow[:, 0:1])
                nc.vector.tensor_reduce(accrow[:, 2:3], siacc[:], axis=AX.X,
                                        op=OP.add)
                nc.vector.tensor_copy(accrow[:, 3:4], scrow[:])
                nc.sync.dma_start(stats_dram[r][:, 0:2], part_i[:])
                nc.sync.dma_start(stats_dram[r][:, 2:4], part_c[:])
                nc.sync.dma_start(stats_dram[r][:, 4:8], accrow[:])

    nc.compile()
    return nc


def _get_nc():
    if "nc" not in _cache:
        _cache["nc"] = _build()
    return _cache["nc"]


def _identity_bf16():
    import ml_dtypes
    return np.eye(128, dtype=ml_dtypes.bfloat16)


def _host_reduce(stats, N, SHARD):
    """stats: [NCORES, P, 128, 8] fp32. Returns reward (float64)."""
    st = stats.astype(np.float64)
    percore = st.sum(axis=2)                 # [NCORES, P, 8]
    kls = []
    for r in range(stats.shape[1]):
        S_i = percore[:, r, 0]
        SS_i = percore[:, r, 1]
        S_c = percore[:, r, 2]
        SS_c = percore[:, r, 3]
        Q = percore[:, r, 4]
        R = percore[:, r, 5]
        Si = percore[:, r, 6]
        Sc = percore[:, r, 7]

        # global stats (ddof=1, + EPS as in reference)
        Sg_i, SSg_i = S_i.sum(), SS_i.sum()
        Sg_c, SSg_c = S_c.sum(), SS_c.sum()
        m_i = Sg_i / N
        s_i = np.sqrt((SSg_i - Sg_i * m_i) / (N - 1)) + EPS
        m_c = Sg_c / N
        s_c = np.sqrt((SSg_c - Sg_c * m_c) / (N - 1)) + EPS

        # per-core local stats (matching device formulas)
        mi_c = S_i / SHARD
        vi_c = (SS_i - S_i * mi_c) / (SHARD - 1)
        si_c = np.sqrt(vi_c)
        mc_c = S_c / SHARD
        vc_c = (SS_c - S_c * mc_c) / (SHARD - 1)
        sc_c = np.sqrt(vc_c)

        ai_c = 1.0 / si_c
        bi_c = -mi_c * ai_c
        QZ = ai_c * Q + bi_c * Si            # sum u*zi_loc per core

        al_i = si_c / s_i                    # zi_glob = al*zi_loc + be
        be_i = (mi_c - m_i) / s_i
        be_c = (mc_c - m_c) / s_c

        eb_i = np.exp(be_i)
        eb_c = np.exp(be_c)

        Si_g = (eb_i * (Si + (al_i - 1.0) * QZ)).sum()
        Sc_g = (eb_c * Sc).sum()
        uz = eb_i * (QZ + (al_i - 1.0) * QZ + be_i * Si)
        uw = eb_i * (R + be_c * Si)
        T = (uz - uw).sum()
        kls.append(T / Si_g + np.log(Sc_g) - np.log(Si_g))
    return -(np.sum(kls) / stats.shape[1])


def kernel(current_params, initial_params):
    from concourse.bass_utils import run_bass_kernel_spmd

    cur = np.asarray(current_params, dtype=np.float32)
    init = np.asarray(initial_params, dtype=np.float32)
    assert cur.shape == (P, N) and init.shape == (P, N)

    nc = _get_nc()
    ident = _identity_bf16()
    in_maps = []
    for c in range(NCORES):
        sl = slice(c * SHARD, (c + 1) * SHARD)
        in_maps.append({
            "xi": init[:, sl].reshape(P, 128, F).copy(),
            "xc": cur[:, sl].reshape(P, 128, F).copy(),
            "ident": ident,
        })
    res = run_bass_kernel_spmd(nc, in_maps, core_ids=list(range(NCORES)))
    _cache["last_results"] = res

    stats = np.stack([res.results[c]["stats"] for c in range(NCORES)])
    return np.float32(_host_reduce(stats, N, SHARD))


# revision 19
# speedup vs baseline: 1.0556x; 1.0556x over previous
"""Trainium2 Bass kernel: parameter-distribution KL (DPO-style) loss.

Computes, for P=4 parameter rows of N=16.7M fp32 elements each:
    z = (x - mean) / std(ddof=1)   per row, both tensors
    p = softmax(z)
    kl_r = sum(p_init * (log p_init - log(p_cur + eps)))
    out = -(sum_r kl_r) / P        (fp32 scalar)

Distribution: flat axis N sharded across 8 NeuronCores, ZERO collectives.
Each core normalizes with its own shard statistics (local mean/std agree
with global to ~5e-4); the host reassembles the global softmax/KL in
float64 with first-order corrections (method error ~1e-6, validated).
Each core reads its HBM shard exactly once and streams continuously.

Device per core, per row (all with LOCAL shard stats):
  cur phase:  bn stats -> a_c,b_c;  w = exp(a_c*xc + b_c) (accum -> Sc)
              wbias = eps*8*Sc_local;  w = ln(w + wbias)   (in-place)
  init phase: bn stats -> a_i,b_i;  u = exp(a_i*xi + b_i) (accum -> Si)
              Q  += diag Gram(u, bf16(xi));  R += diag Gram(u, w)
              (PE, PSUM accumulate; diagonals extracted via identity mask)
Host (float64): per-core sums -> alpha/beta corrections -> global Si, Sc, T;
  kl = T/Si + ln Sc - ln Si.
"""

import numpy as np

P = 4
N = 16777216
NCORES = 8
SHARD = N // NCORES          # 2097152 elements per row per core
F = SHARD // 128             # 16384 free elems per partition
UNITS = 8
EPS = 1e-8

_cache = {}


def _build(F=F, UNITS=UNITS, N=N):
    FU = F // UNITS
    BN_CH = FU // 512
    SH = 128 * F             # local shard size
    import concourse.bacc as bacc
    import concourse.bass_isa as bass_isa
    import concourse.tile as tile
    import concourse.mybir as mybir

    fp32 = mybir.dt.float32
    bf16 = mybir.dt.bfloat16
    AF = mybir.ActivationFunctionType
    OP = mybir.AluOpType
    AX = mybir.AxisListType

    nc = bacc.Bacc("TRN2", target_bir_lowering=False, debug=False,
                   num_devices=NCORES)

    xi_dram = nc.dram_tensor("xi", [P, 128, F], fp32, kind="ExternalInput").ap()
    xc_dram = nc.dram_tensor("xc", [P, 128, F], fp32, kind="ExternalInput").ap()
    id_dram = nc.dram_tensor("ident", [128, 128], bf16,
                             kind="ExternalInput").ap()
    # per row: [128, 12] = [sumi_p, ssqi_p, sumc_p, ssqc_p, q_p, r_p, si_p,
    #                        sc_p, sumi0_p, ssqi0_p, sumc0_p, ssqc0_p]
    # (cols 8-11: unit-0-only partials, the stats the device a/b came from)
    stats_dram = nc.dram_tensor("stats", [P, 128, 12], fp32,
                                kind="ExternalOutput").ap()

    with tile.TileContext(nc) as tc:
        with tc.tile_pool(name="xpool", bufs=8) as xpool, \
             tc.tile_pool(name="bfpool", bufs=3) as bfpool, \
             tc.tile_pool(name="bnpool", bufs=2) as bnpool, \
             tc.tile_pool(name="accpool", bufs=2) as accpool, \
             tc.tile_pool(name="small", bufs=2) as small, \
             tc.tile_pool(name="psum", bufs=2, space="PSUM") as psum:

            ident = small.tile([128, 128], bf16, tag="ident", bufs=1,
                               name="ident")
            nc.sync.dma_start(ident[:], id_dram[:])

            def local_ab(r, x_dram_t, side):
                """Load one tensor of row r, bn stats, local a/b from the
                shard's own statistics. Returns (x_tiles, partials, ab)."""
                x_ts = []
                bn_t = bnpool.tile([128, UNITS * BN_CH, 6], fp32,
                                   tag=f"bn{side}", name=f"bn{side}{r}")
                for k in range(UNITS):
                    x_t = xpool.tile([128, FU], fp32, tag=f"x{side}",
                                     name=f"x{side}{r}_{k}")
                    nc.sync.dma_start(x_t[:], x_dram_t[:, k * FU:(k + 1) * FU])
                    for j in range(BN_CH):
                        idx = k * BN_CH + j
                        nc.vector.bn_stats(bn_t[:, idx:idx + 1, :],
                                           x_t[:, j * 512:(j + 1) * 512])
                    x_ts.append(x_t)
                # full-shard per-partition partials (host output only)
                aggr = small.tile([128, 2], fp32, tag=f"aggr{side}",
                                  name=f"ag{side}{r}")
                nc.vector.bn_aggr(aggr[:], bn_t[:])
                partials = small.tile([128, 2], fp32, tag=f"part{side}",
                                      name=f"pt{side}{r}")
                msq = small.tile([128, 1], fp32, tag=f"msq{side}",
                                 name=f"msq{side}{r}")
                nc.vector.tensor_mul(msq[:], aggr[:, 0:1], aggr[:, 0:1])
                nc.vector.tensor_scalar_mul(partials[:, 0:1], aggr[:, 0:1],
                                            float(F))
                nc.vector.tensor_scalar(partials[:, 1:2], aggr[:, 1:2],
                                        msq[:], float(F),
                                        op0=OP.add, op1=OP.mult)

                # device a/b from UNIT-0 stats only (off the critical path;
                # the host correction handles any local affine, so the exps
                # need not wait for the whole shard's statistics)
                SH0 = 128 * FU
                aggr0 = small.tile([128, 2], fp32, tag=f"aggr0{side}",
                                   name=f"ag0{side}{r}")
                nc.vector.bn_aggr(aggr0[:], bn_t[:, 0:BN_CH, :])
                part0 = small.tile([128, 2], fp32, tag=f"part0{side}",
                                   name=f"pt0{side}{r}")
                msq0 = small.tile([128, 1], fp32, tag=f"msq0{side}",
                                  name=f"msq0{side}{r}")
                nc.vector.tensor_mul(msq0[:], aggr0[:, 0:1], aggr0[:, 0:1])
                nc.vector.tensor_scalar_mul(part0[:, 0:1], aggr0[:, 0:1],
                                            float(FU))
                nc.vector.tensor_scalar(part0[:, 1:2], aggr0[:, 1:2],
                                        msq0[:], float(FU),
                                        op0=OP.add, op1=OP.mult)
                par = small.tile([128, 2], fp32, tag=f"par{side}",
                                 name=f"par{side}{r}")
                nc.gpsimd.partition_all_reduce(par[:], part0[:],
                                               channels=128,
                                               reduce_op=bass_isa.ReduceOp.add)
                # a = var^-0.5 = exp(-0.5*ln var), b = -mean*a
                ab = small.tile([128, 2], fp32, tag=f"ab{side}",
                                name=f"ab{side}{r}")
                tmp = small.tile([128, 3], fp32, tag=f"tmp{side}",
                                 name=f"tm{side}{r}")
                mean, prod, lnv = tmp[:, 0:1], tmp[:, 1:2], tmp[:, 2:3]
                nc.vector.tensor_scalar_mul(mean, par[:, 0:1], 1.0 / SH0)
                nc.vector.tensor_mul(prod, par[:, 0:1], mean)
                nc.vector.tensor_scalar(lnv, par[:, 1:2], prod,
                                        1.0 / (SH0 - 1),
                                        op0=OP.subtract, op1=OP.mult)
                nc.scalar.activation(lnv, lnv, AF.Ln)
                nc.scalar.activation(ab[:, 0:1], lnv, AF.Exp, scale=-0.5)
                nc.vector.tensor_scalar(ab[:, 1:2], ab[:, 0:1], mean, -1.0,
                                        op0=OP.mult, op1=OP.mult)
                return x_ts, partials, part0, ab

            for r in range(P):
                # ---------------- cur phase ----------------
                xc_ts, part_c, p0_c, ab_c = local_ab(r, xc_dram[r], "c")
                a_c, b_c = ab_c[:, 0:1], ab_c[:, 1:2]
                scacc = accpool.tile([128, UNITS], fp32, tag="scacc",
                                     name=f"sc{r}")
                w_ts = []
                for k in range(UNITS):
                    w_t = bfpool.tile([128, FU], bf16, tag="w",
                                      name=f"w{r}_{k}", bufs=12)
                    nc.scalar.activation(w_t[:], xc_ts[k][:], AF.Exp,
                                         bias=b_c, scale=a_c,
                                         accum_out=scacc[:, k:k + 1])
                    w_ts.append(w_t)
                scrow = small.tile([128, 1], fp32, tag="scrow", name=f"scr{r}")
                nc.vector.tensor_reduce(scrow[:], scacc[:], axis=AX.X,
                                        op=OP.add)
                par2 = small.tile([128, 1], fp32, tag="par2", name=f"par2{r}")
                nc.gpsimd.partition_all_reduce(par2[:], scrow[:],
                                               channels=128,
                                               reduce_op=bass_isa.ReduceOp.add)
                wbias = small.tile([128, 1], fp32, tag="wbias", name=f"wb{r}")
                nc.vector.tensor_scalar_mul(wbias[:], par2[:],
                                            EPS * NCORES)
                for k in range(UNITS):
                    nc.scalar.activation(w_ts[k][:], w_ts[k][:], AF.Ln,
                                         bias=wbias[:], scale=1.0)

                # ---------------- init phase ----------------
                xi_ts, part_i, p0_i, ab_i = local_ab(r, xi_dram[r], "i")
                a_i, b_i = ab_i[:, 0:1], ab_i[:, 1:2]
                siacc = accpool.tile([128, UNITS], fp32, tag="siacc",
                                     name=f"si{r}")
                gram_q = psum.tile([128, 128], fp32, tag="gq", name=f"gq{r}")
                gram_r = psum.tile([128, 128], fp32, tag="gr", name=f"gr{r}")
                nchunk = FU // 128
                for k in range(UNITS):
                    u_t = bfpool.tile([128, FU], bf16, tag="u",
                                      name=f"u{r}_{k}", bufs=3)
                    xb_t = bfpool.tile([128, FU], bf16, tag="xb",
                                       name=f"xb{r}_{k}", bufs=3)
                    nc.scalar.activation(u_t[:], xi_ts[k][:], AF.Exp,
                                         bias=b_i, scale=a_i,
                                         accum_out=siacc[:, k:k + 1])
                    nc.vector.tensor_copy(xb_t[:], xi_ts[k][:])
                    for c in range(nchunk):
                        sl = slice(c * 128, (c + 1) * 128)
                        first = (k == 0 and c == 0)
                        last = (k == UNITS - 1 and c == nchunk - 1)
                        nc.tensor.matmul(gram_q[:], u_t[:, sl],
                                         xb_t[:, sl],
                                         start=first, stop=last)
                        nc.tensor.matmul(gram_r[:], u_t[:, sl],
                                         w_ts[k][:, sl],
                                         start=first, stop=last)

                # ---------------- row outputs ----------------
                accrow = accpool.tile([128, 4], fp32, tag="accrow",
                                      name=f"ar{r}")
                dscr = small.tile([128, 128], bf16, tag="dscr", name=f"ds{r}")
                nc.vector.scalar_tensor_tensor(
                    dscr[:], gram_r[:], 1.0, ident[:], OP.mult, OP.mult,
                    accum_out=accrow[:, 1:2])
                dscr2 = small.tile([128, 128], bf16, tag="dscr2",
                                   name=f"ds2{r}")
                nc.vector.scalar_tensor_tensor(
                    dscr2[:], gram_q[:], 1.0, ident[:], OP.mult, OP.mult,
                    accum_out=accrow[:, 0:1])
                nc.vector.tensor_reduce(accrow[:, 2:3], siacc[:], axis=AX.X,
                                        op=OP.add)
                nc.vector.tensor_copy(accrow[:, 3:4], scrow[:])
                nc.sync.dma_start(stats_dram[r][:, 0:2], part_i[:])
                nc.sync.dma_start(stats_dram[r][:, 2:4], part_c[:])
                nc.sync.dma_start(stats_dram[r][:, 4:8], accrow[:])
                nc.sync.dma_start(stats_dram[r][:, 8:10], p0_i[:])
                nc.sync.dma_start(stats_dram[r][:, 10:12], p0_c[:])

    nc.compile()
    return nc


def _get_nc():
    if "nc" not in _cache:
        _cache["nc"] = _build()
    return _cache["nc"]


def _identity_bf16():
    import ml_dtypes
    return np.eye(128, dtype=ml_dtypes.bfloat16)


def _host_reduce(stats, N, SHARD, UNITS=UNITS):
    """stats: [NCORES, P, 128, 12] fp32. Returns reward (float64)."""
    SHARD0 = SHARD // UNITS
    st = stats.astype(np.float64)
    percore = st.sum(axis=2)                 # [NCORES, P, 8]
    kls = []
    for r in range(stats.shape[1]):
        S_i = percore[:, r, 0]
        SS_i = percore[:, r, 1]
        S_c = percore[:, r, 2]
        SS_c = percore[:, r, 3]
        Q = percore[:, r, 4]
        R = percore[:, r, 5]
        Si = percore[:, r, 6]
        Sc = percore[:, r, 7]
        S_i0 = percore[:, r, 8]
        SS_i0 = percore[:, r, 9]
        S_c0 = percore[:, r, 10]
        SS_c0 = percore[:, r, 11]

        # global stats (ddof=1, + EPS as in reference)
        Sg_i, SSg_i = S_i.sum(), SS_i.sum()
        Sg_c, SSg_c = S_c.sum(), SS_c.sum()
        m_i = Sg_i / N
        s_i = np.sqrt((SSg_i - Sg_i * m_i) / (N - 1)) + EPS
        m_c = Sg_c / N
        s_c = np.sqrt((SSg_c - Sg_c * m_c) / (N - 1)) + EPS

        # per-core local affine stats (unit-0 only, matching device)
        mi_c = S_i0 / SHARD0
        vi_c = (SS_i0 - S_i0 * mi_c) / (SHARD0 - 1)
        si_c = np.sqrt(vi_c)
        mc_c = S_c0 / SHARD0
        vc_c = (SS_c0 - S_c0 * mc_c) / (SHARD0 - 1)
        sc_c = np.sqrt(vc_c)

        ai_c = 1.0 / si_c
        bi_c = -mi_c * ai_c
        QZ = ai_c * Q + bi_c * Si            # sum u*zi_loc per core

        al_i = si_c / s_i                    # zi_glob = al*zi_loc + be
        be_i = (mi_c - m_i) / s_i
        be_c = (mc_c - m_c) / s_c

        eb_i = np.exp(be_i)
        eb_c = np.exp(be_c)

        Si_g = (eb_i * (Si + (al_i - 1.0) * QZ)).sum()
        Sc_g = (eb_c * Sc).sum()
        uz = eb_i * (QZ + (al_i - 1.0) * QZ + be_i * Si)
        uw = eb_i * (R + be_c * Si)
        T = (uz - uw).sum()
        kls.append(T / Si_g + np.log(Sc_g) - np.log(Si_g))
    return -(np.sum(kls) / stats.shape[1])


def kernel(current_params, initial_params):
    from concourse.bass_utils import run_bass_kernel_spmd

    cur = np.asarray(current_params, dtype=np.float32)
    init = np.asarray(initial_params, dtype=np.float32)
    assert cur.shape == (P, N) and init.shape == (P, N)

    nc = _get_nc()
    ident = _identity_bf16()
    in_maps = []
    for c in range(NCORES):
        sl = slice(c * SHARD, (c + 1) * SHARD)
        in_maps.append({
            "xi": init[:, sl].reshape(P, 128, F).copy(),
            "xc": cur[:, sl].reshape(P, 128, F).copy(),
            "ident": ident,
        })
    res = run_bass_kernel_spmd(nc, in_maps, core_ids=list(range(NCORES)))
    _cache["last_results"] = res

    stats = np.stack([res.results[c]["stats"] for c in range(NCORES)])
    return np.float32(_host_reduce(stats, N, SHARD))
